# revision 14
# baseline (speedup 1.0000x reference)
"""3-layer GAT + per-graph mean-pool + linear head, distributed over 8 NeuronCores.

v2 strategy (edge-parallel, dst-sorted, bf16 tables, batched Ant-gathers):
  * Host: sort edges by dst; core c owns dst range [c*2560, (c+1)*2560) =
    20 windows of 128 dst nodes.  Window edge lists padded (src=0,
    dstloc=300) to nblk*128 slots (nblk = global max, SPMD-uniform).
  * Per layer a DRAM table ztab[l] [N, 384] bf16 holds rows
    [z(256) | el(4) | er(4) | pad(120)]; row stride 768B (%256 ok).
    Layer 0's table is computed fully replicated on every core; layers
    1-2 compute the local 2560-row slice (compact [2560, 264]) and
    AllGather into ztab[:, 0:264].
  * Edge phase per window: gather z-rows for all edge slots with 2-3
    dma_gather (InstDMAGatherAnt) instructions (<=1024 int16 indices
    each, rotating over 4 SWDGE queues) -- ~2.5-4 ns/descriptor vs
    ~10 ns/desc for per-block indirect DMA.  er[dst] is expanded
    edge-wise on PE: per 128-edge block, sel = one-hot(dstloc) (DVE
    is_equal, bf16), selT = PE transpose, er_mm = selT.T @ erw_own.
    Softmax: ex = exp(leaky(el+er)) written back into the el slot so the
    scatter matmul rhs = [z*ex | ex]; per-block scatter (lhsT=sel)
    accumulates [out | sum_ex] in PSUM f32.  Normalize after
    aggregation (softmax is shift-invariant; logits can't overflow exp).
  * elu chains run mostly on the idle Scalar engine:
    elu(x)      = max(x,0) + exp(min(x,0)) - 1
    elu(elu(x)) = max(x,0) + exp(exp(min(x,0)) - 1) - 1
    with min(x,0) = -Relu(-x) (ACT), exps on ACT, one DVE STT each.
  * Pooling: per-window graph-membership one-hot matmul (bf16) accumulates
    graph sums; each core emits logits for its own 8 graphs; host concats.
"""

import sys

import ml_dtypes
import numpy as np

sys.path.insert(0, "/opt/trn_rl_repo")

import concourse.bass as bass
import concourse.bacc as bacc
import concourse.mybir as mybir
import concourse.tile as tile
from concourse.bass_utils import run_bass_kernel_spmd
from concourse.masks import make_identity

# Problem shape (hardcoded per contest rules).
N, E, G = 20480, 327680, 64
IN_DIM, H, D, C = 128, 4, 64, 10
HD = H * D            # 256
ROW = HD + 2 * H      # 264 = z | el | er (compact, for slices/AllGather)
ROWP = 384            # padded table row; 768 bytes (bf16), %256 == 0
NCORES = 8
RN = N // NCORES      # 2560 dst nodes per core
P = 128
NW = RN // P          # 20 windows per core
G8 = G // NCORES      # 8 graphs per core
NEG_SLOPE = 0.2
F32 = mybir.dt.float32
BF16 = mybir.dt.bfloat16
I32 = mybir.dt.int32
I16 = mybir.dt.int16
AF = mybir.ActivationFunctionType

TRACE = False         # set by test.py to capture HW profile
LAST_EXEC_NS = None
LAST_RESULTS = None

_CACHE = {}


def _install_ntff_hook_shim():
    """This image's ``antenv`` lacks ``axon_hooks``; provide the thin ctypes
    shim around libaxon_pjrt.so so run_bass_kernel_spmd(trace=True) works."""
    try:
        import antenv.axon_hooks  # noqa: F401
        return
    except ImportError:
        pass
    import contextlib
    import ctypes
    import types

    so_path = "/opt/axon/libaxon_pjrt.so"
    try:
        lib = ctypes.CDLL(so_path)
    except OSError:
        return
    if not hasattr(lib, "axon_start_nrt_profile"):
        return
    lib.axon_start_nrt_profile.argtypes = [ctypes.POINTER(ctypes.c_int64), ctypes.c_size_t]
    lib.axon_start_nrt_profile.restype = ctypes.c_int64
    lib.axon_stop_nrt_profile.argtypes = [ctypes.c_char_p]
    lib.axon_stop_nrt_profile.restype = ctypes.c_int64

    @contextlib.contextmanager
    def _hook(output_dir, device_ids):
        import jax

        jax.devices()
        if device_ids:
            ids = (ctypes.c_int64 * len(device_ids))(*device_ids)
            rc = lib.axon_start_nrt_profile(ids, len(device_ids))
        else:
            rc = lib.axon_start_nrt_profile(None, 0)
        if rc != 0:
            raise RuntimeError(f"axon_start_nrt_profile rc={rc}")
        try:
            yield
        finally:
            n = lib.axon_stop_nrt_profile(str(output_dir).encode())
            print(f"ntff profile: {n} file(s) written to {output_dir}")

    mod = types.ModuleType("antenv.axon_hooks")
    mod.get_axon_ntff_profile_hook = lambda: _hook
    mod.set_axon_ntff_profile_hook = lambda h: None
    sys.modules["antenv.axon_hooks"] = mod


# ----------------------------------------------------------------------------
# Host-side index preprocessing (layout only -- no arithmetic on tensor data)
# ----------------------------------------------------------------------------
def _host_prep(src, dst, graph_ids):
    order = np.argsort(dst, kind="stable")
    src_s = src[order].astype(np.int64)
    dst_s = dst[order].astype(np.int64)
    win = dst_s // P                              # global window 0..159
    cnt = np.bincount(win, minlength=NCORES * NW)
    nblk = int(np.ceil(cnt.max() / P))
    slots = nblk * P

    starts = np.zeros(NCORES * NW, np.int64)
    starts[1:] = np.cumsum(cnt)[:-1]
    srcidx = np.zeros((NCORES * NW, slots), np.int16)              # pad -> row 0
    dstloc = np.full((NCORES * NW, slots), 300.0, np.float32)      # pad -> no match
    for w in range(NCORES * NW):
        c0, c1 = starts[w], starts[w] + cnt[w]
        srcidx[w, : cnt[w]] = src_s[c0:c1]
        dstloc[w, : cnt[w]] = (dst_s[c0:c1] - w * P).astype(np.float32)

    # table-row permutation: AllGather is chunked 4-way; chunk q of core c
    # lands at rows [q*5120 + c*640, +640).  perm(n) maps node id -> table row.
    nn = np.arange(N, dtype=np.int64)
    perm = (nn % 2560 // 640) * (NCORES * 640) + (nn // 2560) * 640 + nn % 640
    srcidx = perm[srcidx].astype(np.int16)

    # wrapped int16 index layout for dma_gather: slot i -> (part i%16, col i//16),
    # replicated over the 8 Q7 cores (partitions 16..127)
    IW = slots // 16
    srci_d, dstl_d = [], []
    for c in range(NCORES):
        wrap = np.zeros((16, NW * IW), np.int16)
        for w in range(NW):
            a = srcidx[c * NW + w].reshape(IW, 16).T       # (i%16, i//16)
            wrap[:, w * IW:(w + 1) * IW] = a
        srci_d.append(np.ascontiguousarray(np.tile(wrap, (8, 1))))
        # dstloc per-slot in (p, w*nblk+b) layout, edge slot = b*128+p
        a = dstloc[c * NW: (c + 1) * NW].reshape(NW, nblk, P)
        a = np.transpose(a, (2, 0, 1)).reshape(P, NW * nblk)
        dstl_d.append(np.ascontiguousarray(a.astype(np.float32)))

    gids = np.asarray(graph_ids).astype(np.int64).reshape(NCORES, NW, P)
    gmask = []
    for c in range(NCORES):
        m = np.zeros((P, NW * G8), np.float32)
        for w in range(NW):
            loc = gids[c, w] - c * G8              # 0..7 within this core
            m[np.arange(P), w * G8 + loc] = 1.0
        gmask.append(np.ascontiguousarray(m.astype(ml_dtypes.bfloat16)))
    return nblk, srci_d, dstl_d, gmask


def _blockdiag(a):
    # [H, D] -> [HD, H] block-diagonal layout so  el = z @ a_bd
    out = np.zeros((HD, H), np.float32)
    for h in range(H):
        out[h * D: (h + 1) * D, h] = a[h]
    return out


# ----------------------------------------------------------------------------
# Device program
# ----------------------------------------------------------------------------
def _build_program(nblk):
    slots = nblk * P
    IW = slots // 16
    # gather chunks: <=1024 idxs per dma_gather, multiples of 128
    chunks = []
    c0 = 0
    while c0 < slots:
        csz = min(1024, slots - c0)
        chunks.append((c0, csz))
        c0 += csz

    nc = bacc.Bacc(
        "TRN2",
        target_bir_lowering=False,
        debug=False,
        enable_asserts=False,
        num_devices=NCORES,
        num_swdge_queues=4,
    )

    xT = nc.dram_tensor("xT", [IN_DIM, N], F32, kind="ExternalInput")
    Ws, WTs, ALs, ARs = [], [], [], []
    for l, K in enumerate([IN_DIM, HD, HD]):
        Ws.append(nc.dram_tensor(f"W{l}", [K, HD], F32, kind="ExternalInput"))
        WTs.append(nc.dram_tensor(f"WT{l}", [HD, K], F32, kind="ExternalInput"))
        ALs.append(nc.dram_tensor(f"albd{l}", [HD, H], F32, kind="ExternalInput"))
        ARs.append(nc.dram_tensor(f"arbd{l}", [HD, H], F32, kind="ExternalInput"))
    Wc = nc.dram_tensor("Wc", [HD, C], F32, kind="ExternalInput")
    bc = nc.dram_tensor("bc_rep", [G8, C], F32, kind="ExternalInput")
    srci = nc.dram_tensor("srcidx", [P, NW * IW], I16, kind="ExternalInput")
    dstl = nc.dram_tensor("dstloc", [P, NW * nblk], F32, kind="ExternalInput")
    gmk = nc.dram_tensor("gmask", [P, NW * G8], BF16, kind="ExternalInput")
    logits = nc.dram_tensor("logits", [G8, C], F32, kind="ExternalOutput")

    ztab = [nc.dram_tensor(f"ztab{l}", [N, ROWP], BF16) for l in range(3)]
    NCH = 4                        # AllGather chunks per layer
    CHW = NW // NCH                # windows per chunk (5)
    zsl = [None,
           [nc.dram_tensor(f"zsl1_{q}", [CHW * P, ROWP], BF16) for q in range(NCH)],
           [nc.dram_tensor(f"zsl2_{q}", [CHW * P, ROWP], BF16) for q in range(NCH)]]

    AL = mybir.AluOpType

    with tile.TileContext(nc) as tc:
        with (
            tc.tile_pool(name="const", bufs=1) as constp,
            tc.tile_pool(name="wload", bufs=2) as wloadp,
            tc.tile_pool(name="mm", bufs=3) as mmp,
            tc.tile_pool(name="edge", bufs=6) as edgep,
            tc.tile_pool(name="sel", bufs=2) as selp,
            tc.tile_pool(name="selt", bufs=3) as seltp,
            tc.tile_pool(name="small", bufs=4) as smallp,
            tc.tile_pool(name="psmm", bufs=3, space="PSUM") as psmm,
            tc.tile_pool(name="psout", bufs=2, space="PSUM") as psout,
            tc.tile_pool(name="pser", bufs=2, space="PSUM") as pser,
            tc.tile_pool(name="pshg", bufs=1, space="PSUM") as pshg,
        ):
            # ---- constants / resident state ----
            ident_f = constp.tile([P, P], F32, tag="ident_f")
            make_identity(nc, ident_f[:])
            ident_b = constp.tile([P, P], BF16, tag="ident_b")
            nc.vector.tensor_copy(ident_b[:], ident_f[:])
            iota_i = constp.tile([P, P], I32, tag="iota_i")
            nc.gpsimd.iota(iota_i[:], pattern=[[1, P]], base=0, channel_multiplier=0)
            iota_b = constp.tile([P, 1, P], BF16, tag="iota_b")
            nc.vector.tensor_copy(iota_b[:, 0, :], iota_i[:])
            srci_sb = constp.tile([P, NW * IW], I16, tag="srci")
            nc.sync.dma_start(srci_sb[:], srci[:, :])
            dstl_sb = constp.tile([P, NW * nblk], F32, tag="dstl")
            nc.sync.dma_start(dstl_sb[:], dstl[:, :])
            dstl_b = constp.tile([P, NW * nblk], BF16, tag="dstl_b")
            nc.vector.tensor_copy(dstl_b[:], dstl_sb[:])
            gmk_sb = constp.tile([P, NW * G8], BF16, tag="gmk")
            nc.sync.dma_start(gmk_sb[:], gmk[:, :])
            h_all = constp.tile([P, NW, HD], F32, tag="h_all")
            neg1 = constp.tile([P, 1], F32, tag="neg1")
            nc.gpsimd.memset(neg1[:], -1.0)
            hg_acc = constp.tile([G8, HD], F32, tag="hg_acc")
            nc.gpsimd.memset(hg_acc[:], 0.0)
            # own-slice el|er per layer (erw[l][:, w, 0:8]; er at cols 4:8)
            erw = [constp.tile([P, NW, 2 * H], BF16, tag=f"erw{l}", name=f"erw{l}")
                   for l in range(3)]

            def build_wext(l, K):
                """wext[l]: [kch][128, ROW] bf16 = [W | W@albd | W@arbd]."""
                kch = K // P
                och = HD // P
                W_sb, WT_sb, al_sb, ar_sb = [], [], [], []
                for k in range(kch):
                    t = wloadp.tile([P, HD], F32, tag="wld")
                    nc.sync.dma_start(t[:], Ws[l][k * P: (k + 1) * P, :])
                    W_sb.append(t)
                for oc in range(och):
                    t = wloadp.tile([P, K], F32, tag="wtld")
                    nc.sync.dma_start(t[:], WTs[l][oc * P: (oc + 1) * P, :])
                    WT_sb.append(t)
                    ta = wloadp.tile([P, H], F32, tag="alld")
                    nc.sync.dma_start(ta[:], ALs[l][oc * P: (oc + 1) * P, :])
                    al_sb.append(ta)
                    tr = wloadp.tile([P, H], F32, tag="arld")
                    nc.sync.dma_start(tr[:], ARs[l][oc * P: (oc + 1) * P, :])
                    ar_sb.append(tr)
                wext = []
                for k in range(kch):
                    wx = constp.tile([P, ROW], BF16, tag=f"wext{l}_{k}")
                    nc.vector.tensor_copy(wx[:, 0:HD], W_sb[k][:])
                    for dstcol, bd in ((HD, al_sb), (HD + H, ar_sb)):
                        ps = psmm.tile([P, H], F32, tag="mm")
                        for oc in range(och):
                            nc.tensor.matmul(
                                ps[:],
                                lhsT=WT_sb[oc][:, k * P: (k + 1) * P],
                                rhs=bd[oc][:],
                                start=(oc == 0),
                                stop=(oc == och - 1),
                            )
                        nc.vector.tensor_copy(wx[:, dstcol: dstcol + H], ps[:])
                    wext.append(wx)
                return wext

            def l0_table(wext):
                """Full replicated table for layer 0 (x is available on every
                core, so no collective); own el|er is fetched afterwards by
                own_elr() via a small dma_gather of the core's node range.
                Batched 5 chunks per DMA (sync engine issue rate), written to
                the permuted row layout (640-node groups stay contiguous)."""
                TB = 5
                for t0 in range(0, N // P, TB):
                    n0 = t0 * P
                    row0 = (n0 % 2560 // 640) * (NCORES * 640) \
                        + (n0 // 2560) * 640 + n0 % 640
                    xt = mmp.tile([P, TB * P], F32, tag="xt")
                    nc.sync.dma_start(xt[:], xT[:, n0: n0 + TB * P])
                    xb = mmp.tile([P, TB * P], BF16, tag="xb")
                    nc.scalar.copy(xb[:], xt[:])
                    zsb = mmp.tile([P, TB, ROW], BF16, tag="zsb")
                    for tb in range(TB):
                        zp = psmm.tile([P, ROW], F32, tag="mm")
                        nc.tensor.matmul(zp[:], lhsT=xb[:, tb * P: (tb + 1) * P],
                                         rhs=wext[0][:], start=True, stop=True)
                        if tb % 2 == 0:
                            nc.scalar.copy(zsb[:, tb, :], zp[:])
                        else:
                            nc.vector.tensor_copy(zsb[:, tb, :], zp[:])
                    out_ap = ztab[0][row0: row0 + TB * P, 0:ROW].rearrange(
                        "(t p) r -> p t r", t=TB)
                    nc.sync.dma_start(out_ap, zsb[:])

            ownod = nc.dram_tensor("ownod", [P, RN // 16], I16, kind="ExternalInput")
            ownod_sb = constp.tile([P, RN // 16], I16, tag="ownod")
            nc.sync.dma_start(ownod_sb[:], ownod[:, :])

            def own_elr(l):
                """Gather own rows' el|er cols from ztab[l] into erw[l]."""
                # 2560 idxs -> 3 chunks; elem = 128 cols (256B) at col 256
                g = smallp.tile([P, RN // P, P], BF16, tag="ownelr")
                cc = 0
                j = 0
                while cc < RN:
                    csz = min(1024, RN - cc)
                    nc.gpsimd.dma_gather(
                        g[:, cc // P: (cc + csz) // P, :],
                        ztab[l][:, HD:ROWP],
                        ownod_sb[:, cc // 16: (cc + csz) // 16],
                        csz, csz, ROWP - HD,
                        elem_step=ROWP,
                        queue_num=j % 4,
                    )
                    cc += csz
                    j += 1
                # own node (w*128+p) sits at g[p, w, :]; el|er at cols 0:8
                nc.vector.tensor_copy(erw[l][:], g[:, :, 0: 2 * H])

            def slice_row(l, w, hb):
                """z-table slice row for layer l (1 or 2), window w, from the
                just-computed bf16 activations hb; issues the chunk AllGather
                after the last window of each quarter."""
                hts = []
                for c2 in range(2):
                    tp = psmm.tile([P, P], BF16, tag="mm")
                    nc.tensor.transpose(
                        tp[:], hb[:, c2 * P: (c2 + 1) * P], ident_b[:]
                    )
                    ht = mmp.tile([P, P], BF16, tag="ht")
                    nc.vector.tensor_copy(ht[:], tp[:])
                    hts.append(ht)
                zp = psmm.tile([P, ROW], F32, tag="mm")
                for c2 in range(2):
                    nc.tensor.matmul(
                        zp[:],
                        lhsT=hts[c2][:],
                        rhs=wexts[l][c2][:],
                        start=(c2 == 0),
                        stop=(c2 == 1),
                    )
                zs = mmp.tile([P, ROW], BF16, tag="zs")
                nc.scalar.copy(zs[:], zp[:])
                nc.vector.tensor_copy(erw[l][:, w, :], zs[:, HD:ROW])
                q, wq = w // CHW, w % CHW
                nc.sync.dma_start(
                    zsl[l][q][wq * P: (wq + 1) * P, 0:ROW], zs[:])
                if wq == CHW - 1:
                    nc.gpsimd.collective_compute(
                        "AllGather",
                        AL.bypass,
                        replica_groups=[list(range(NCORES))],
                        ins=[zsl[l][q][:, :]],
                        outs=[ztab[l][q * NCORES * CHW * P:
                                      (q + 1) * NCORES * CHW * P, :]],
                    )

            def edge_phase(l):
                # gathers are issued PF windows ahead so the AllGather triggers
                # (also on the gpsimd queue) never stall pending gather issues
                PF = 4
                pend = {}

                def issue_gathers(w):
                    zel = edgep.tile([P, nblk, ROWP], BF16, tag="zel")
                    for j, (cc, csz) in enumerate(chunks):
                        nc.gpsimd.dma_gather(
                            zel[:, cc // P: (cc + csz) // P, :],
                            ztab[l][:, :],
                            srci_sb[:, w * IW + cc // 16: w * IW + (cc + csz) // 16],
                            csz, csz, ROWP,
                            queue_num=(w * len(chunks) + j) % 4,
                        )
                    pend[w] = zel

                for w in range(min(PF, NW)):
                    issue_gathers(w)
                for w in range(NW):
                    zel = pend.pop(w)
                    if w + PF < NW:
                        issue_gathers(w + PF)
                    # whole-window one-hot sel (one DVE op), then per-block
                    # er[dst] expansion on PE (selT = transpose(sel_b))
                    sel_win = selp.tile([P, nblk, P], BF16, tag="selw")
                    nc.vector.tensor_tensor(
                        out=sel_win[:],
                        in0=dstl_b[:, w * nblk: (w + 1) * nblk].to_broadcast(
                            [P, nblk, P]),
                        in1=iota_b[:].to_broadcast([P, nblk, P]),
                        op=AL.is_equal,
                    )
                    er_ps = pser.tile([P, nblk, H], F32, tag="erps")
                    for b in range(nblk):
                        stp = psmm.tile([P, P], BF16, tag="mm")
                        nc.tensor.transpose(stp[:], sel_win[:, b, :], ident_b[:])
                        selt = seltp.tile([P, P], BF16, tag="selt")
                        if b % 2 == 0:
                            nc.vector.tensor_copy(selt[:], stp[:])
                        else:
                            nc.scalar.copy(selt[:], stp[:])
                        nc.tensor.matmul(
                            er_ps[:, b, :], lhsT=selt[:],
                            rhs=erw[l][:, w, H: 2 * H],
                            start=True, stop=True,
                        )
                    # e = el[src] + er[dst]; ex = exp(leaky(e)) -> el slot
                    eall = smallp.tile([P, nblk, H], BF16, tag="eall")
                    nc.vector.tensor_tensor(
                        out=eall[:], in0=zel[:, :, HD: HD + H],
                        in1=er_ps[:], op=AL.add,
                    )
                    nc.vector.scalar_tensor_tensor(
                        out=eall[:], in0=eall[:], scalar=NEG_SLOPE,
                        in1=eall[:], op0=AL.mult, op1=AL.max,
                    )
                    nc.scalar.activation(zel[:, :, HD: HD + H], eall[:], AF.Exp)
                    # scale messages in place: z *= ex (per-head broadcast)
                    zb = zel[:, :, 0:HD].rearrange("p b (h d) -> p b h d", h=H)
                    nc.vector.tensor_tensor(
                        out=zb, in0=zb,
                        in1=zel[:, :, HD: HD + H].to_broadcast([P, nblk, H, D]),
                        op=AL.mult,
                    )
                    # scatter-add via one-hot matmuls: [out | sum_ex] in PSUM
                    outp = psout.tile([P, HD + H], F32, tag="outp")
                    for b in range(nblk):
                        nc.tensor.matmul(
                            outp[:],
                            lhsT=sel_win[:, b, :],
                            rhs=zel[:, b, 0: HD + H],
                            start=(b == 0),
                            stop=(b == nblk - 1),
                        )
                    # normalize + residual + elu chain
                    rec = smallp.tile([P, H], F32, tag="rec")
                    nc.vector.reciprocal(rec[:], outp[:, HD: HD + H])
                    agg = mmp.tile([P, HD], F32, tag="agg")
                    nc.vector.tensor_tensor(
                        out=agg[:].rearrange("p (h d) -> p h d", h=H),
                        in0=outp[:, 0:HD].rearrange("p (h d) -> p h d", h=H),
                        in1=rec[:].to_broadcast([P, H, D]),
                        op=AL.mult,
                    )
                    if l > 0:
                        nc.vector.tensor_tensor(
                            out=agg[:], in0=agg[:], in1=h_all[:, w, :], op=AL.add
                        )
                    # single elu (l=0) or double elu (l>0), exp work on ACT:
                    # r = Relu(-x); e1 = Exp(-r) = exp(min(x,0))
                    r = mmp.tile([P, HD], F32, tag="elur")
                    nc.scalar.activation(r[:], agg[:], AF.Relu, scale=-1.0)
                    e1 = mmp.tile([P, HD], F32, tag="elue")
                    nc.scalar.activation(e1[:], r[:], AF.Exp, scale=-1.0)
                    if l > 0:
                        e2 = mmp.tile([P, HD], F32, tag="elue2")
                        nc.scalar.activation(e2[:], e1[:], AF.Exp, bias=neg1[:])
                        e1 = e2
                    # v = max(x,0) + e; h = v - 1
                    v = mmp.tile([P, HD], F32, tag="eluv")
                    nc.vector.scalar_tensor_tensor(
                        out=v[:], in0=agg[:], scalar=0.0, in1=e1[:],
                        op0=AL.max, op1=AL.add,
                    )
                    nc.scalar.activation(h_all[:, w, :], v[:], AF.Copy, bias=-1.0)
                    hb = mmp.tile([P, HD], BF16, tag="hb")
                    nc.vector.tensor_copy(hb[:], h_all[:, w, :])
                    if l < 2:
                        slice_row(l + 1, w, hb)
                    else:
                        gp = pshg.tile([G8, HD], F32, tag="hg")
                        nc.tensor.matmul(
                            gp[:],
                            lhsT=gmk_sb[:, w * G8: (w + 1) * G8],
                            rhs=hb[:],
                            start=True,
                            stop=True,
                        )
                        nc.vector.tensor_tensor(
                            out=hg_acc[:], in0=hg_acc[:], in1=gp[:], op=AL.add
                        )

            # ---- build all weight tables upfront ----
            wexts = [build_wext(0, IN_DIM), build_wext(1, HD), build_wext(2, HD)]
            # ---- layer 0 ----
            l0_table(wexts[0])
            own_elr(0)
            edge_phase(0)
            # ---- layers 1, 2 ----
            for l in (1, 2):
                edge_phase(l)

            # ---- pooling epilogue: hg -> elu -> @Wc + bc ----
            hg_sb = smallp.tile([G8, HD], F32, tag="hg_sb")
            nc.vector.tensor_scalar_mul(hg_sb[:], hg_acc[:], 1.0 / (N // G))
            mn = smallp.tile([G8, HD], F32, tag="fmn")
            nc.vector.tensor_scalar_min(mn[:], hg_sb[:], 0.0)
            exx = smallp.tile([G8, HD], F32, tag="fex")
            nc.scalar.activation(exx[:], mn[:], AF.Exp)
            mx = smallp.tile([G8, HD], F32, tag="fmx")
            nc.vector.tensor_scalar_max(mx[:], hg_sb[:], 0.0)
            nc.vector.tensor_scalar_add(exx[:], exx[:], -1.0)
            nc.vector.tensor_tensor(out=hg_sb[:], in0=exx[:], in1=mx[:], op=AL.add)

            wc_sb, hgts = [], []
            for c2 in range(2):
                t = smallp.tile([P, C], F32, tag="wc")
                nc.sync.dma_start(t[:], Wc[c2 * P: (c2 + 1) * P, :])
                wc_sb.append(t)
                tp = psmm.tile([P, G8], F32, tag="mm")
                nc.tensor.transpose(
                    tp[:], hg_sb[:, c2 * P: (c2 + 1) * P], ident_f[:G8, :G8]
                )
                hgt = smallp.tile([P, G8], F32, tag="hgt")
                nc.vector.tensor_copy(hgt[:], tp[:])
                hgts.append(hgt)
            lg = psmm.tile([G8, C], F32, tag="mm")
            for c2 in range(2):
                nc.tensor.matmul(
                    lg[:], lhsT=hgts[c2][:], rhs=wc_sb[c2][:],
                    start=(c2 == 0), stop=(c2 == 1),
                )
            bc_sb = smallp.tile([G8, C], F32, tag="bc")
            nc.sync.dma_start(bc_sb[:], bc[:, :])
            lg_sb = smallp.tile([G8, C], F32, tag="lg")
            nc.vector.tensor_tensor(out=lg_sb[:], in0=lg[:], in1=bc_sb[:], op=AL.add)
            nc.sync.dma_start(logits[:, :], lg_sb[:])

    nc.compile()
    return nc


def _get_program(nblk):
    if nblk not in _CACHE:
        _CACHE[nblk] = _build_program(nblk)
    return _CACHE[nblk]


# ----------------------------------------------------------------------------
# Entry point
# ----------------------------------------------------------------------------
def kernel(x, src, dst, graph_ids, W0, al0, ar0, W1, al1, ar1, W2, al2, ar2, Wc, bc):
    global LAST_EXEC_NS, LAST_RESULTS
    x = np.ascontiguousarray(np.asarray(x, np.float32))
    src = np.asarray(src).astype(np.int32)
    dst = np.asarray(dst).astype(np.int32)
    graph_ids = np.asarray(graph_ids).astype(np.int32)

    nblk, srci_d, dstl_d, gmask_d = _host_prep(src, dst, graph_ids)
    nc = _get_program(nblk)

    xT = np.ascontiguousarray(x.T)
    Wl = [np.asarray(W0, np.float32), np.asarray(W1, np.float32), np.asarray(W2, np.float32)]
    als = [al0, al1, al2]
    ars = [ar0, ar1, ar2]
    common = {"xT": xT, "Wc": np.asarray(Wc, np.float32),
              "bc_rep": np.tile(np.asarray(bc, np.float32)[None, :], (G8, 1))}
    for l in range(3):
        common[f"W{l}"] = Wl[l]
        common[f"WT{l}"] = np.ascontiguousarray(Wl[l].T)
        common[f"albd{l}"] = _blockdiag(np.asarray(als[l], np.float32))
        common[f"arbd{l}"] = _blockdiag(np.asarray(ars[l], np.float32))

    in_maps = []
    for c in range(NCORES):
        m = dict(common)
        m["srcidx"] = srci_d[c]
        m["dstloc"] = dstl_d[c]
        m["gmask"] = gmask_d[c]
        nn = np.arange(N, dtype=np.int64)
        perm = (nn % 2560 // 640) * (NCORES * 640) + (nn // 2560) * 640 + nn % 640
        own = perm[c * RN + np.arange(RN)].astype(np.int16)
        wrap = np.ascontiguousarray(own.reshape(RN // 16, 16).T)
        m["ownod"] = np.ascontiguousarray(np.tile(wrap, (8, 1)))
        in_maps.append(m)

    if TRACE:
        _install_ntff_hook_shim()
    res = run_bass_kernel_spmd(nc, in_maps, list(range(NCORES)), trace=TRACE)
    LAST_EXEC_NS = res.exec_time_ns
    LAST_RESULTS = res
    out = np.concatenate([res.results[c]["logits"] for c in range(NCORES)], axis=0)
    return out.astype(np.float32)


# revision 18
# speedup vs baseline: 1.3245x; 1.3245x over previous
"""3-layer GAT + per-graph mean-pool + linear head, distributed over 8 NeuronCores.

v2 strategy (edge-parallel, dst-sorted, bf16 tables, batched Ant-gathers):
  * Host: sort edges by dst; core c owns dst range [c*2560, (c+1)*2560) =
    20 windows of 128 dst nodes.  Window edge lists padded (src=0,
    dstloc=300) to nblk*128 slots (nblk = global max, SPMD-uniform).
  * Per layer a DRAM table ztab[l] [N, 384] bf16 holds rows
    [z(256) | el(4) | er(4) | pad(120)]; row stride 768B (%256 ok).
    Layer 0's table is computed fully replicated on every core; layers
    1-2 compute the local 2560-row slice (compact [2560, 264]) and
    AllGather into ztab[:, 0:264].
  * Edge phase per window: gather z-rows for all edge slots with 2-3
    dma_gather (InstDMAGatherAnt) instructions (<=1024 int16 indices
    each, rotating over 4 SWDGE queues) -- ~2.5-4 ns/descriptor vs
    ~10 ns/desc for per-block indirect DMA.  er[dst] is expanded
    edge-wise on PE: per 128-edge block, sel = one-hot(dstloc) (DVE
    is_equal, bf16), selT = PE transpose, er_mm = selT.T @ erw_own.
    Softmax: ex = exp(leaky(el+er)) written back into the el slot so the
    scatter matmul rhs = [z*ex | ex]; per-block scatter (lhsT=sel)
    accumulates [out | sum_ex] in PSUM f32.  Normalize after
    aggregation (softmax is shift-invariant; logits can't overflow exp).
  * elu chains run mostly on the idle Scalar engine:
    elu(x)      = max(x,0) + exp(min(x,0)) - 1
    elu(elu(x)) = max(x,0) + exp(exp(min(x,0)) - 1) - 1
    with min(x,0) = -Relu(-x) (ACT), exps on ACT, one DVE STT each.
  * Pooling: per-window graph-membership one-hot matmul (bf16) accumulates
    graph sums; each core emits logits for its own 8 graphs; host concats.
"""

import sys

import ml_dtypes
import numpy as np

sys.path.insert(0, "/opt/trn_rl_repo")

import concourse.bass as bass
import concourse.bacc as bacc
import concourse.mybir as mybir
import concourse.tile as tile
from concourse.bass_utils import run_bass_kernel_spmd
from concourse.masks import make_identity

# Problem shape (hardcoded per contest rules).
N, E, G = 20480, 327680, 64
IN_DIM, H, D, C = 128, 4, 64, 10
HD = H * D            # 256
ROW = HD + 2 * H      # 264 = z | el | er (compact psum width)
ROWB = 512            # table row BYTES: z fp8(256) | el/er bf16(16) | pad
NCORES = 8
RN = N // NCORES      # 2560 dst nodes per core
P = 128
NW = RN // P          # 20 windows per core
G8 = G // NCORES      # 8 graphs per core
NEG_SLOPE = 0.2
F32 = mybir.dt.float32
BF16 = mybir.dt.bfloat16
FP8 = mybir.dt.float8e4
I32 = mybir.dt.int32
I16 = mybir.dt.int16
AF = mybir.ActivationFunctionType

TRACE = False         # set by test.py to capture HW profile
LAST_EXEC_NS = None
LAST_RESULTS = None

_CACHE = {}


def _install_ntff_hook_shim():
    """This image's ``antenv`` lacks ``axon_hooks``; provide the thin ctypes
    shim around libaxon_pjrt.so so run_bass_kernel_spmd(trace=True) works."""
    try:
        import antenv.axon_hooks  # noqa: F401
        return
    except ImportError:
        pass
    import contextlib
    import ctypes
    import types

    so_path = "/opt/axon/libaxon_pjrt.so"
    try:
        lib = ctypes.CDLL(so_path)
    except OSError:
        return
    if not hasattr(lib, "axon_start_nrt_profile"):
        return
    lib.axon_start_nrt_profile.argtypes = [ctypes.POINTER(ctypes.c_int64), ctypes.c_size_t]
    lib.axon_start_nrt_profile.restype = ctypes.c_int64
    lib.axon_stop_nrt_profile.argtypes = [ctypes.c_char_p]
    lib.axon_stop_nrt_profile.restype = ctypes.c_int64

    @contextlib.contextmanager
    def _hook(output_dir, device_ids):
        import jax

        jax.devices()
        if device_ids:
            ids = (ctypes.c_int64 * len(device_ids))(*device_ids)
            rc = lib.axon_start_nrt_profile(ids, len(device_ids))
        else:
            rc = lib.axon_start_nrt_profile(None, 0)
        if rc != 0:
            raise RuntimeError(f"axon_start_nrt_profile rc={rc}")
        try:
            yield
        finally:
            n = lib.axon_stop_nrt_profile(str(output_dir).encode())
            print(f"ntff profile: {n} file(s) written to {output_dir}")

    mod = types.ModuleType("antenv.axon_hooks")
    mod.get_axon_ntff_profile_hook = lambda: _hook
    mod.set_axon_ntff_profile_hook = lambda h: None
    sys.modules["antenv.axon_hooks"] = mod


# ----------------------------------------------------------------------------
# Host-side index preprocessing (layout only -- no arithmetic on tensor data)
# ----------------------------------------------------------------------------
def _host_prep(src, dst, graph_ids):
    order = np.argsort(dst, kind="stable")
    src_s = src[order].astype(np.int64)
    dst_s = dst[order].astype(np.int64)
    win = dst_s // P                              # global window 0..159
    cnt = np.bincount(win, minlength=NCORES * NW)
    nblk = int(np.ceil(cnt.max() / P))
    slots = nblk * P

    starts = np.zeros(NCORES * NW, np.int64)
    starts[1:] = np.cumsum(cnt)[:-1]
    srcidx = np.zeros((NCORES * NW, slots), np.int16)              # pad -> row 0
    dstloc = np.full((NCORES * NW, slots), 300.0, np.float32)      # pad -> no match
    for w in range(NCORES * NW):
        c0, c1 = starts[w], starts[w] + cnt[w]
        srcidx[w, : cnt[w]] = src_s[c0:c1]
        dstloc[w, : cnt[w]] = (dst_s[c0:c1] - w * P).astype(np.float32)

    # table-row permutation: AllGather is chunked 4-way; chunk q of core c
    # lands at rows [q*5120 + c*640, +640).  perm(n) maps node id -> table row.
    nn = np.arange(N, dtype=np.int64)
    perm = (nn % 2560 // 640) * (NCORES * 640) + (nn // 2560) * 640 + nn % 640
    srcidx = perm[srcidx].astype(np.int16)

    # wrapped int16 index layout for dma_gather: slot i -> (part i%16, col i//16),
    # replicated over the 8 Q7 cores (partitions 16..127)
    IW = slots // 16
    srci_d, dstl_d = [], []
    for c in range(NCORES):
        wrap = np.zeros((16, NW * IW), np.int16)
        for w in range(NW):
            a = srcidx[c * NW + w].reshape(IW, 16).T       # (i%16, i//16)
            wrap[:, w * IW:(w + 1) * IW] = a
        srci_d.append(np.ascontiguousarray(np.tile(wrap, (8, 1))))
        # dstloc per-slot in (p, w*nblk+b) layout, edge slot = b*128+p
        a = dstloc[c * NW: (c + 1) * NW].reshape(NW, nblk, P)
        a = np.transpose(a, (2, 0, 1)).reshape(P, NW * nblk)
        dstl_d.append(np.ascontiguousarray(a.astype(np.float32)))

    gids = np.asarray(graph_ids).astype(np.int64).reshape(NCORES, NW, P)
    gmask = []
    for c in range(NCORES):
        m = np.zeros((P, NW * G8), np.float32)
        for w in range(NW):
            loc = gids[c, w] - c * G8              # 0..7 within this core
            m[np.arange(P), w * G8 + loc] = 1.0
        gmask.append(np.ascontiguousarray(m.astype(ml_dtypes.bfloat16)))
    return nblk, srci_d, dstl_d, gmask


def _blockdiag(a):
    # [H, D] -> [HD, H] block-diagonal layout so  el = z @ a_bd
    out = np.zeros((HD, H), np.float32)
    for h in range(H):
        out[h * D: (h + 1) * D, h] = a[h]
    return out


# ----------------------------------------------------------------------------
# Device program
# ----------------------------------------------------------------------------
def _build_program(nblk):
    slots = nblk * P
    IW = slots // 16
    # gather chunks: <=1024 idxs per dma_gather, multiples of 128
    chunks = []
    c0 = 0
    while c0 < slots:
        csz = min(1024, slots - c0)
        chunks.append((c0, csz))
        c0 += csz

    nc = bacc.Bacc(
        "TRN2",
        target_bir_lowering=False,
        debug=False,
        enable_asserts=False,
        num_devices=NCORES,
        num_swdge_queues=4,
    )

    xT = nc.dram_tensor("xT", [IN_DIM, N], F32, kind="ExternalInput")
    Ws, WTs, ALs, ARs = [], [], [], []
    for l, K in enumerate([IN_DIM, HD, HD]):
        Ws.append(nc.dram_tensor(f"W{l}", [K, HD], F32, kind="ExternalInput"))
        WTs.append(nc.dram_tensor(f"WT{l}", [HD, K], F32, kind="ExternalInput"))
        ALs.append(nc.dram_tensor(f"albd{l}", [HD, H], F32, kind="ExternalInput"))
        ARs.append(nc.dram_tensor(f"arbd{l}", [HD, H], F32, kind="ExternalInput"))
    Wc = nc.dram_tensor("Wc", [HD, C], F32, kind="ExternalInput")
    bc = nc.dram_tensor("bc_rep", [G8, C], F32, kind="ExternalInput")
    srci = nc.dram_tensor("srcidx", [P, NW * IW], I16, kind="ExternalInput")
    dstl = nc.dram_tensor("dstloc", [P, NW * nblk], F32, kind="ExternalInput")
    gmk = nc.dram_tensor("gmask", [P, NW * G8], BF16, kind="ExternalInput")
    logits = nc.dram_tensor("logits", [G8, C], F32, kind="ExternalOutput")

    ztab = [nc.dram_tensor(f"ztab{l}", [N, ROWB], FP8) for l in range(3)]
    NCH = 4                        # AllGather chunks per layer
    CHW = NW // NCH                # windows per chunk (5)
    zsl = [None,
           [nc.dram_tensor(f"zsl1_{q}", [CHW * P, ROWB], FP8) for q in range(NCH)],
           [nc.dram_tensor(f"zsl2_{q}", [CHW * P, ROWB], FP8) for q in range(NCH)]]

    AL = mybir.AluOpType

    with tile.TileContext(nc) as tc:
        with (
            tc.tile_pool(name="const", bufs=1) as constp,
            tc.tile_pool(name="wload", bufs=2) as wloadp,
            tc.tile_pool(name="mm", bufs=3) as mmp,
            tc.tile_pool(name="edge", bufs=4) as edgep,
            tc.tile_pool(name="msg", bufs=2) as msgp,
            tc.tile_pool(name="sel", bufs=2) as selp,
            tc.tile_pool(name="selt", bufs=3) as seltp,
            tc.tile_pool(name="small", bufs=4) as smallp,
            tc.tile_pool(name="psmm", bufs=3, space="PSUM") as psmm,
            tc.tile_pool(name="psout", bufs=2, space="PSUM") as psout,
            tc.tile_pool(name="pser", bufs=2, space="PSUM") as pser,
            tc.tile_pool(name="pshg", bufs=1, space="PSUM") as pshg,
        ):
            # ---- constants / resident state ----
            ident_f = constp.tile([P, P], F32, tag="ident_f")
            make_identity(nc, ident_f[:])
            ident_b = constp.tile([P, P], BF16, tag="ident_b")
            nc.vector.tensor_copy(ident_b[:], ident_f[:])
            iota_i = constp.tile([P, P], I32, tag="iota_i")
            nc.gpsimd.iota(iota_i[:], pattern=[[1, P]], base=0, channel_multiplier=0)
            iota_b = constp.tile([P, 1, P], BF16, tag="iota_b")
            nc.vector.tensor_copy(iota_b[:, 0, :], iota_i[:])
            srci_sb = constp.tile([P, NW * IW], I16, tag="srci")
            nc.sync.dma_start(srci_sb[:], srci[:, :])
            dstl_sb = constp.tile([P, NW * nblk], F32, tag="dstl")
            nc.sync.dma_start(dstl_sb[:], dstl[:, :])
            dstl_b = constp.tile([P, NW * nblk], BF16, tag="dstl_b")
            nc.vector.tensor_copy(dstl_b[:], dstl_sb[:])
            gmk_sb = constp.tile([P, NW * G8], BF16, tag="gmk")
            nc.sync.dma_start(gmk_sb[:], gmk[:, :])
            h_all = constp.tile([P, NW, HD], F32, tag="h_all")
            neg1 = constp.tile([P, 1], F32, tag="neg1")
            nc.gpsimd.memset(neg1[:], -1.0)
            hg_acc = constp.tile([G8, HD], F32, tag="hg_acc")
            nc.gpsimd.memset(hg_acc[:], 0.0)
            # own-slice el|er per layer (erw[l][:, w, 0:8]; er at cols 4:8)
            erw = [constp.tile([P, NW, 2 * H], BF16, tag=f"erw{l}", name=f"erw{l}")
                   for l in range(3)]

            def build_wext(l, K):
                """wext[l]: [kch][128, ROW] bf16 = [W | W@albd | W@arbd]."""
                kch = K // P
                och = HD // P
                W_sb, WT_sb, al_sb, ar_sb = [], [], [], []
                for k in range(kch):
                    t = wloadp.tile([P, HD], F32, tag="wld")
                    nc.sync.dma_start(t[:], Ws[l][k * P: (k + 1) * P, :])
                    W_sb.append(t)
                for oc in range(och):
                    t = wloadp.tile([P, K], F32, tag="wtld")
                    nc.sync.dma_start(t[:], WTs[l][oc * P: (oc + 1) * P, :])
                    WT_sb.append(t)
                    ta = wloadp.tile([P, H], F32, tag="alld")
                    nc.sync.dma_start(ta[:], ALs[l][oc * P: (oc + 1) * P, :])
                    al_sb.append(ta)
                    tr = wloadp.tile([P, H], F32, tag="arld")
                    nc.sync.dma_start(tr[:], ARs[l][oc * P: (oc + 1) * P, :])
                    ar_sb.append(tr)
                wext = []
                for k in range(kch):
                    wx = constp.tile([P, ROW], BF16, tag=f"wext{l}_{k}")
                    nc.vector.tensor_copy(wx[:, 0:HD], W_sb[k][:])
                    for dstcol, bd in ((HD, al_sb), (HD + H, ar_sb)):
                        ps = psmm.tile([P, H], F32, tag="mm")
                        for oc in range(och):
                            nc.tensor.matmul(
                                ps[:],
                                lhsT=WT_sb[oc][:, k * P: (k + 1) * P],
                                rhs=bd[oc][:],
                                start=(oc == 0),
                                stop=(oc == och - 1),
                            )
                        nc.vector.tensor_copy(wx[:, dstcol: dstcol + H], ps[:])
                    wext.append(wx)
                return wext

            def l0_table(wext):
                """Full replicated table for layer 0 (x is available on every
                core, so no collective); own el|er is fetched afterwards by
                own_elr() via a small dma_gather of the core's node range.
                Batched 5 chunks per DMA (sync engine issue rate), written to
                the permuted row layout (640-node groups stay contiguous)."""
                TB = 5
                for t0 in range(0, N // P, TB):
                    n0 = t0 * P
                    row0 = (n0 % 2560 // 640) * (NCORES * 640) \
                        + (n0 // 2560) * 640 + n0 % 640
                    xt = mmp.tile([P, TB * P], F32, tag="xt")
                    nc.sync.dma_start(xt[:], xT[:, n0: n0 + TB * P])
                    xb = mmp.tile([P, TB * P], BF16, tag="xb")
                    nc.scalar.copy(xb[:], xt[:])
                    zsb = mmp.tile([P, TB, HD], FP8, tag="zsb")
                    zse = mmp.tile([P, TB, 2 * H], BF16, tag="zse")
                    for tb in range(TB):
                        zp = psmm.tile([P, ROW], F32, tag="mm")
                        nc.tensor.matmul(zp[:], lhsT=xb[:, tb * P: (tb + 1) * P],
                                         rhs=wext[0][:], start=True, stop=True)
                        if tb % 2 == 0:
                            nc.scalar.copy(zsb[:, tb, :], zp[:, 0:HD])
                        else:
                            nc.vector.tensor_copy(zsb[:, tb, :], zp[:, 0:HD])
                        nc.vector.tensor_copy(zse[:, tb, :], zp[:, HD:ROW])
                    nc.sync.dma_start(
                        ztab[0][row0: row0 + TB * P, 0:HD].rearrange(
                            "(t p) r -> p t r", t=TB), zsb[:])
                    nc.sync.dma_start(
                        ztab[0][row0: row0 + TB * P, HD: HD + 4 * H].bitcast(
                            BF16).rearrange("(t p) r -> p t r", t=TB), zse[:])

            ownod = nc.dram_tensor("ownod", [P, RN // 16], I16, kind="ExternalInput")
            ownod_sb = constp.tile([P, RN // 16], I16, tag="ownod")
            nc.sync.dma_start(ownod_sb[:], ownod[:, :])

            def own_elr(l):
                """Gather own rows' el|er cols from ztab[l] into erw[l]."""
                # 2560 idxs -> 3 chunks; elem = 128 cols (256B) at col 256
                g = smallp.tile([P, RN // P, HD], FP8, tag="ownelr")
                cc = 0
                j = 0
                while cc < RN:
                    csz = min(1024, RN - cc)
                    nc.gpsimd.dma_gather(
                        g[:, cc // P: (cc + csz) // P, :],
                        ztab[l][:, HD:ROWB],
                        ownod_sb[:, cc // 16: (cc + csz) // 16],
                        csz, csz, ROWB - HD,
                        elem_step=ROWB,
                        queue_num=j % 4,
                    )
                    cc += csz
                    j += 1
                # own node (w*128+p) sits at g[p, w, :]; el|er bf16 at bytes 0:16
                nc.vector.tensor_copy(erw[l][:], g[:, :, 0: 4 * H].bitcast(BF16))

            def slice_row(l, w, hb):
                """z-table slice row for layer l (1 or 2), window w, from the
                just-computed bf16 activations hb; issues the chunk AllGather
                after the last window of each quarter."""
                hts = []
                for c2 in range(2):
                    tp = psmm.tile([P, P], BF16, tag="mm")
                    nc.tensor.transpose(
                        tp[:], hb[:, c2 * P: (c2 + 1) * P], ident_b[:]
                    )
                    ht = mmp.tile([P, P], BF16, tag="ht")
                    nc.vector.tensor_copy(ht[:], tp[:])
                    hts.append(ht)
                zp = psmm.tile([P, ROW], F32, tag="mm")
                for c2 in range(2):
                    nc.tensor.matmul(
                        zp[:],
                        lhsT=hts[c2][:],
                        rhs=wexts[l][c2][:],
                        start=(c2 == 0),
                        stop=(c2 == 1),
                    )
                zs = mmp.tile([P, HD], FP8, tag="zs")
                nc.scalar.copy(zs[:], zp[:, 0:HD])
                zse = mmp.tile([P, 2 * H], BF16, tag="zse")
                nc.vector.tensor_copy(zse[:], zp[:, HD:ROW])
                nc.vector.tensor_copy(erw[l][:, w, :], zse[:])
                q, wq = w // CHW, w % CHW
                nc.sync.dma_start(
                    zsl[l][q][wq * P: (wq + 1) * P, 0:HD], zs[:])
                nc.sync.dma_start(
                    zsl[l][q][wq * P: (wq + 1) * P, HD: HD + 4 * H].bitcast(BF16),
                    zse[:])
                if wq == CHW - 1:
                    nc.gpsimd.collective_compute(
                        "AllGather",
                        AL.bypass,
                        replica_groups=[list(range(NCORES))],
                        ins=[zsl[l][q][:, :]],
                        outs=[ztab[l][q * NCORES * CHW * P:
                                      (q + 1) * NCORES * CHW * P, :]],
                    )

            def edge_phase(l):
                # gathers are issued PF windows ahead so the AllGather triggers
                # (also on the gpsimd queue) never stall pending gather issues
                PF = 2
                pend = {}

                def issue_gathers(w):
                    zel = edgep.tile([P, nblk, ROWB], FP8, tag="zel")
                    for j, (cc, csz) in enumerate(chunks):
                        nc.gpsimd.dma_gather(
                            zel[:, cc // P: (cc + csz) // P, :],
                            ztab[l][:, :],
                            srci_sb[:, w * IW + cc // 16: w * IW + (cc + csz) // 16],
                            csz, csz, ROWB,
                            queue_num=(w * len(chunks) + j) % 4,
                        )
                    pend[w] = zel

                for w in range(min(PF, NW)):
                    issue_gathers(w)
                for w in range(NW):
                    zel = pend.pop(w)
                    if w + PF < NW:
                        issue_gathers(w + PF)
                    # whole-window one-hot sel (one DVE op), then per-block
                    # er[dst] expansion on PE (selT = transpose(sel_b))
                    sel_win = selp.tile([P, nblk, P], BF16, tag="selw")
                    nc.vector.tensor_tensor(
                        out=sel_win[:],
                        in0=dstl_b[:, w * nblk: (w + 1) * nblk].to_broadcast(
                            [P, nblk, P]),
                        in1=iota_b[:].to_broadcast([P, nblk, P]),
                        op=AL.is_equal,
                    )
                    er_ps = pser.tile([P, nblk, H], F32, tag="erps")
                    for b in range(nblk):
                        stp = psmm.tile([P, P], BF16, tag="mm")
                        nc.tensor.transpose(stp[:], sel_win[:, b, :], ident_b[:])
                        selt = seltp.tile([P, P], BF16, tag="selt")
                        if b % 2 == 0:
                            nc.vector.tensor_copy(selt[:], stp[:])
                        else:
                            nc.scalar.copy(selt[:], stp[:])
                        nc.tensor.matmul(
                            er_ps[:, b, :], lhsT=selt[:],
                            rhs=erw[l][:, w, H: 2 * H],
                            start=True, stop=True,
                        )
                    # e = el[src] + er[dst]; ex = exp(leaky(e)) -> msg ex slot
                    elv = zel[:, :, HD: HD + 2 * H].bitcast(BF16)   # [P,nblk,4]
                    eall = smallp.tile([P, nblk, H], BF16, tag="eall")
                    nc.vector.tensor_tensor(
                        out=eall[:], in0=elv, in1=er_ps[:], op=AL.add,
                    )
                    nc.vector.scalar_tensor_tensor(
                        out=eall[:], in0=eall[:], scalar=NEG_SLOPE,
                        in1=eall[:], op0=AL.mult, op1=AL.max,
                    )
                    msg = msgp.tile([P, nblk, HD + H], BF16, tag="msg")
                    nc.scalar.activation(msg[:, :, HD: HD + H], eall[:], AF.Exp)
                    # messages: msg_z = z_fp8 * ex (per-head broadcast)
                    nc.vector.tensor_tensor(
                        out=msg[:, :, 0:HD].rearrange("p b (h d) -> p b h d", h=H),
                        in0=zel[:, :, 0:HD].rearrange("p b (h d) -> p b h d", h=H),
                        in1=msg[:, :, HD: HD + H].to_broadcast([P, nblk, H, D]),
                        op=AL.mult,
                    )
                    # scatter-add via one-hot matmuls: [out | sum_ex] in PSUM
                    outp = psout.tile([P, HD + H], F32, tag="outp")
                    for b in range(nblk):
                        nc.tensor.matmul(
                            outp[:],
                            lhsT=sel_win[:, b, :],
                            rhs=msg[:, b, 0: HD + H],
                            start=(b == 0),
                            stop=(b == nblk - 1),
                        )
                    # normalize + residual + elu chain
                    rec = smallp.tile([P, H], F32, tag="rec")
                    nc.vector.reciprocal(rec[:], outp[:, HD: HD + H])
                    agg = mmp.tile([P, HD], F32, tag="agg")
                    nc.vector.tensor_tensor(
                        out=agg[:].rearrange("p (h d) -> p h d", h=H),
                        in0=outp[:, 0:HD].rearrange("p (h d) -> p h d", h=H),
                        in1=rec[:].to_broadcast([P, H, D]),
                        op=AL.mult,
                    )
                    if l > 0:
                        nc.vector.tensor_tensor(
                            out=agg[:], in0=agg[:], in1=h_all[:, w, :], op=AL.add
                        )
                    # single elu (l=0) or double elu (l>0), exp work on ACT:
                    # r = Relu(-x); e1 = Exp(-r) = exp(min(x,0))
                    r = mmp.tile([P, HD], F32, tag="elur")
                    nc.scalar.activation(r[:], agg[:], AF.Relu, scale=-1.0)
                    e1 = mmp.tile([P, HD], F32, tag="elue")
                    nc.scalar.activation(e1[:], r[:], AF.Exp, scale=-1.0)
                    if l > 0:
                        e2 = mmp.tile([P, HD], F32, tag="elue2")
                        nc.scalar.activation(e2[:], e1[:], AF.Exp, bias=neg1[:])
                        e1 = e2
                    # v = max(x,0) + e; h = v - 1
                    v = mmp.tile([P, HD], F32, tag="eluv")
                    nc.vector.scalar_tensor_tensor(
                        out=v[:], in0=agg[:], scalar=0.0, in1=e1[:],
                        op0=AL.max, op1=AL.add,
                    )
                    nc.scalar.activation(h_all[:, w, :], v[:], AF.Copy, bias=-1.0)
                    hb = mmp.tile([P, HD], BF16, tag="hb")
                    nc.vector.tensor_copy(hb[:], h_all[:, w, :])
                    if l < 2:
                        slice_row(l + 1, w, hb)
                    else:
                        gp = pshg.tile([G8, HD], F32, tag="hg")
                        nc.tensor.matmul(
                            gp[:],
                            lhsT=gmk_sb[:, w * G8: (w + 1) * G8],
                            rhs=hb[:],
                            start=True,
                            stop=True,
                        )
                        nc.vector.tensor_tensor(
                            out=hg_acc[:], in0=hg_acc[:], in1=gp[:], op=AL.add
                        )

            # ---- build all weight tables upfront ----
            wexts = [build_wext(0, IN_DIM), build_wext(1, HD), build_wext(2, HD)]
            # ---- layer 0 ----
            l0_table(wexts[0])
            own_elr(0)
            edge_phase(0)
            # ---- layers 1, 2 ----
            for l in (1, 2):
                edge_phase(l)

            # ---- pooling epilogue: hg -> elu -> @Wc + bc ----
            hg_sb = smallp.tile([G8, HD], F32, tag="hg_sb")
            nc.vector.tensor_scalar_mul(hg_sb[:], hg_acc[:], 1.0 / (N // G))
            mn = smallp.tile([G8, HD], F32, tag="fmn")
            nc.vector.tensor_scalar_min(mn[:], hg_sb[:], 0.0)
            exx = smallp.tile([G8, HD], F32, tag="fex")
            nc.scalar.activation(exx[:], mn[:], AF.Exp)
            mx = smallp.tile([G8, HD], F32, tag="fmx")
            nc.vector.tensor_scalar_max(mx[:], hg_sb[:], 0.0)
            nc.vector.tensor_scalar_add(exx[:], exx[:], -1.0)
            nc.vector.tensor_tensor(out=hg_sb[:], in0=exx[:], in1=mx[:], op=AL.add)

            wc_sb, hgts = [], []
            for c2 in range(2):
                t = smallp.tile([P, C], F32, tag="wc")
                nc.sync.dma_start(t[:], Wc[c2 * P: (c2 + 1) * P, :])
                wc_sb.append(t)
                tp = psmm.tile([P, G8], F32, tag="mm")
                nc.tensor.transpose(
                    tp[:], hg_sb[:, c2 * P: (c2 + 1) * P], ident_f[:G8, :G8]
                )
                hgt = smallp.tile([P, G8], F32, tag="hgt")
                nc.vector.tensor_copy(hgt[:], tp[:])
                hgts.append(hgt)
            lg = psmm.tile([G8, C], F32, tag="mm")
            for c2 in range(2):
                nc.tensor.matmul(
                    lg[:], lhsT=hgts[c2][:], rhs=wc_sb[c2][:],
                    start=(c2 == 0), stop=(c2 == 1),
                )
            bc_sb = smallp.tile([G8, C], F32, tag="bc")
            nc.sync.dma_start(bc_sb[:], bc[:, :])
            lg_sb = smallp.tile([G8, C], F32, tag="lg")
            nc.vector.tensor_tensor(out=lg_sb[:], in0=lg[:], in1=bc_sb[:], op=AL.add)
            nc.sync.dma_start(logits[:, :], lg_sb[:])

    nc.compile()
    return nc


def _get_program(nblk):
    if nblk not in _CACHE:
        _CACHE[nblk] = _build_program(nblk)
    return _CACHE[nblk]


# ----------------------------------------------------------------------------
# Entry point
# ----------------------------------------------------------------------------
def kernel(x, src, dst, graph_ids, W0, al0, ar0, W1, al1, ar1, W2, al2, ar2, Wc, bc):
    global LAST_EXEC_NS, LAST_RESULTS
    x = np.ascontiguousarray(np.asarray(x, np.float32))
    src = np.asarray(src).astype(np.int32)
    dst = np.asarray(dst).astype(np.int32)
    graph_ids = np.asarray(graph_ids).astype(np.int32)

    nblk, srci_d, dstl_d, gmask_d = _host_prep(src, dst, graph_ids)
    nc = _get_program(nblk)

    xT = np.ascontiguousarray(x.T)
    Wl = [np.asarray(W0, np.float32), np.asarray(W1, np.float32), np.asarray(W2, np.float32)]
    als = [al0, al1, al2]
    ars = [ar0, ar1, ar2]
    common = {"xT": xT, "Wc": np.asarray(Wc, np.float32),
              "bc_rep": np.tile(np.asarray(bc, np.float32)[None, :], (G8, 1))}
    for l in range(3):
        common[f"W{l}"] = Wl[l]
        common[f"WT{l}"] = np.ascontiguousarray(Wl[l].T)
        common[f"albd{l}"] = _blockdiag(np.asarray(als[l], np.float32))
        common[f"arbd{l}"] = _blockdiag(np.asarray(ars[l], np.float32))

    in_maps = []
    for c in range(NCORES):
        m = dict(common)
        m["srcidx"] = srci_d[c]
        m["dstloc"] = dstl_d[c]
        m["gmask"] = gmask_d[c]
        nn = np.arange(N, dtype=np.int64)
        perm = (nn % 2560 // 640) * (NCORES * 640) + (nn // 2560) * 640 + nn % 640
        own = perm[c * RN + np.arange(RN)].astype(np.int16)
        wrap = np.ascontiguousarray(own.reshape(RN // 16, 16).T)
        m["ownod"] = np.ascontiguousarray(np.tile(wrap, (8, 1)))
        in_maps.append(m)

    if TRACE:
        _install_ntff_hook_shim()
    res = run_bass_kernel_spmd(nc, in_maps, list(range(NCORES)), trace=TRACE)
    LAST_EXEC_NS = res.exec_time_ns
    LAST_RESULTS = res
    out = np.concatenate([res.results[c]["logits"] for c in range(NCORES)], axis=0)
    return out.astype(np.float32)
